# revision 3
# baseline (speedup 1.0000x reference)
"""Inclusive prefix-sum (Blelloch scan, additive) along L for X_in (8, 4096, 64, 16) f32.

Sharding: batch B=8 across the 8 NeuronCores (one batch per core; no communication).
Per core the problem is a cumsum along L=4096 of a (L, F=1024) matrix.

Per-core kernel ("transposed-output matmul scan", bf16 I/O):
  - The rel-err gate is 2e-2, so device I/O is bf16 (host converts f32<->bf16):
    16 MiB of HBM traffic per core instead of 32 MiB -> ~47 us DMA floor.
  - L is processed in M=2-block (256-row) windows. Per window and 128-wide
    feature group, 3 bf16 matmuls accumulate into one PSUM tile [128, 256]:
        mm0: lhsT=x_b0, rhs=[triu|ones]  -> cols 0..255   (start)
        mm1: lhsT=x_b1, rhs= triu        -> cols 128..255 (accumulate, stop)
    giving the within-window inclusive scan transposed (features on partitions),
    accumulated in f32 PSUM.
  - The inter-window carry is a per-partition scalar fused into the PSUM->SBUF
    copy (DVE tensor_scalar_add for groups 0-3, ACT activation bias for groups
    4-7), which also converts f32->bf16. Carry for window w is the last
    already-written column of the staged bf16 output; window 0 uses a zeros
    column so every window runs the identical op.
  - Input DMA: 1 MiB chunks (4 L-blocks) on the sync HWDGE ring. Output staged
    in (128, 2048) bf16 tiles -> 512 KiB out-DMAs, also on sync.
  - Output is written transposed, y (F, L) bf16; numpy un-transposes and
    upcasts when unsharding.
"""

import numpy as np

B, L, D, N = 8, 4096, 64, 16
F = D * N            # 1024 features per batch
NCORES = 8
LBLK = 128           # L positions per matmul block
M = 2                # L-blocks per PSUM window (vector-op granularity)
NGROUP = F // 128    # 8 feature groups
NBLK = L // LBLK     # 32 L-blocks
NWIN = NBLK // M     # 16 windows
CBLK = 4             # L-blocks per input DMA chunk (1 MiB)
SPAN = 2048          # L columns per staged output tile (512 KiB out-DMAs)
WIN_PER_SPAN = SPAN // (M * LBLK)
XIN_BUFS = 4

_CACHE = {}


def _build_nc(loop_nrep=None):
    """Build the Bass program. loop_nrep wraps the body in a device-side For_i -
    used only by test.py for timing (the graded path uses loop_nrep=None)."""
    from contextlib import nullcontext

    import concourse.bacc as bacc
    import concourse.mybir as mybir
    from concourse.tile import TileContext

    f32 = mybir.dt.float32
    bf16 = mybir.dt.bfloat16
    nc = bacc.Bacc(
        "TRN2", target_bir_lowering=False, debug=False, num_devices=NCORES
    )
    x = nc.dram_tensor("x", (L, F), bf16, kind="ExternalInput")
    u = nc.dram_tensor("u", (LBLK, M * LBLK), bf16, kind="ExternalInput")
    y = nc.dram_tensor("y", (F, L), bf16, kind="ExternalOutput")

    with TileContext(nc) as tc:
        with (
            tc.tile_pool(name="const", bufs=1) as cpool,
            tc.tile_pool(name="xin", bufs=XIN_BUFS) as xpool,
            tc.tile_pool(name="stage", bufs=2) as spool,
            tc.tile_pool(name="psum", bufs=8, space="PSUM") as ppool,
        ):
            ut = cpool.tile([LBLK, M * LBLK], bf16)
            nc.sync.dma_start(out=ut[:], in_=u[:, :])
            zt = cpool.tile([128, 1], bf16)
            nc.vector.memset(zt[:], 0.0)

            loop_cm = tc.For_i(0, loop_nrep, 1) if loop_nrep else nullcontext()
            loop_cm.__enter__()
            staged = [None] * NGROUP
            prev_staged = [None] * NGROUP
            for ci in range(NBLK // CBLK):  # 1 MiB input chunks: 4 L-blocks
                xt = xpool.tile([128, CBLK * F], bf16, tag="xt", name=f"xt_{ci}")
                nc.sync.dma_start(
                    out=xt[:],
                    in_=x[ci * (CBLK * 128) : (ci + 1) * (CBLK * 128), :].rearrange(
                        "(t p) f -> p t f", p=128
                    ),
                )
                for wi in range(CBLK // M):
                    w = ci * (CBLK // M) + wi
                    s, wb = divmod(w, WIN_PER_SPAN)
                    for g in range(NGROUP):
                        if wb == 0:
                            prev_staged[g] = staged[g]
                            staged[g] = spool.tile(
                                [128, SPAN], bf16, tag=f"st{g}", name=f"st{g}_{s}"
                            )
                        ps = ppool.tile(
                            [128, M * LBLK], f32, tag="ps", name=f"ps_{w}_{g}"
                        )
                        for j in range(M):
                            nc.tensor.matmul(
                                ps[:, j * LBLK :],
                                xt[
                                    :,
                                    (wi * M + j) * F
                                    + g * 128 : (wi * M + j) * F
                                    + (g + 1) * 128,
                                ],
                                ut[:, : (M - j) * LBLK],
                                start=(j == 0),
                                stop=(j == M - 1),
                            )
                        dst = staged[g][:, wb * M * LBLK : (wb + 1) * M * LBLK]
                        if w == 0:
                            carry = zt[:]
                        elif wb > 0:
                            carry = staged[g][:, wb * M * LBLK - 1 : wb * M * LBLK]
                        else:
                            carry = prev_staged[g][:, SPAN - 1 : SPAN]
                        if g < NGROUP // 2:
                            nc.vector.tensor_tensor(
                                out=dst,
                                in0=ps[:],
                                in1=carry.broadcast_to([128, M * LBLK]),
                                op=mybir.AluOpType.add,
                            )
                        else:
                            nc.scalar.add(out=dst, in_=ps[:], add=carry)
                        if wb == WIN_PER_SPAN - 1:
                            nc.sync.dma_start(
                                out=y[
                                    g * 128 : (g + 1) * 128, s * SPAN : (s + 1) * SPAN
                                ],
                                in_=staged[g][:],
                            )
            loop_cm.__exit__(None, None, None)
    nc.compile()
    return nc


def _get_nc():
    if "nc" not in _CACHE:
        _CACHE["nc"] = _build_nc()
    return _CACHE["nc"]


def _make_in_maps(X_in):
    import ml_dtypes

    bf16 = ml_dtypes.bfloat16
    xs = np.ascontiguousarray(np.asarray(X_in, dtype=np.float32)).reshape(B, L, F)
    xs = xs.astype(bf16)
    umat = np.concatenate(
        [np.triu(np.ones((LBLK, LBLK), dtype=np.float32))]
        + [np.ones((LBLK, LBLK), dtype=np.float32)] * (M - 1),
        axis=1,
    ).astype(bf16)
    return [{"x": xs[b], "u": umat} for b in range(B)]


def _unshard(per_core_outs):
    out = np.empty((B, L, D, N), dtype=np.float32)
    for b in range(B):
        out[b] = np.asarray(per_core_outs[b]["y"], dtype=np.float32).T.reshape(L, D, N)
    return out


def kernel(X_in):
    from concourse.bass_utils import run_bass_kernel_spmd

    nc = _get_nc()
    res = run_bass_kernel_spmd(nc, _make_in_maps(X_in), core_ids=list(range(NCORES)))
    return _unshard(res.results)


# revision 10
# speedup vs baseline: 1.0215x; 1.0215x over previous
"""Inclusive prefix-sum (Blelloch scan, additive) along L for X_in (8, 4096, 64, 16) f32.

Sharding: batch B=8 across the 8 NeuronCores (one batch per core; no communication).
Per core the problem is a cumsum along L=4096 of a (L, F=1024) matrix.

Per-core kernel ("transposed-output matmul scan", bf16 I/O):
  - The rel-err gate is 2e-2, so device I/O is bf16 (host converts f32<->bf16):
    16 MiB of HBM traffic per core instead of 32 MiB -> ~47 us DMA floor.
  - L is processed in M=2-block (256-row) windows. Per window and 128-wide
    feature group, 3 bf16 matmuls accumulate into one PSUM tile [128, 256]:
        mm0: lhsT=x_b0, rhs=[triu|ones]  -> cols 0..255   (start)
        mm1: lhsT=x_b1, rhs= triu        -> cols 128..255 (accumulate, stop)
    giving the within-window inclusive scan transposed (features on partitions),
    accumulated in f32 PSUM.
  - The inter-window carry is a per-partition scalar fused into the PSUM->SBUF
    copy (DVE tensor_tensor with a stride-0 broadcast carry for groups 0-3 —
    DVE tensor_scalar requires f32 scalars — ACT activation bias for groups
    4-7), which also converts f32->bf16. Carry for window w is the last
    already-written column of the staged bf16 output; window 0 uses a zeros
    column so every window runs the identical op.
  - Input DMA: 512 KiB chunks (2 L-blocks) on the sync HWDGE ring (smaller
    first-chunk ramp measured ~0.8 us faster than 1 MiB chunks). Output staged
    in (128, 2048) bf16 tiles -> 512 KiB out-DMAs, also on sync.
  - Output is written transposed, y (F, L) bf16; numpy un-transposes and
    upcasts when unsharding.

Measured (For_i loop-diff on HW, 8 cores concurrent): ~61 us/iteration incl.
~13.5 us For_i back-edge; ~47.4 us single-shot, vs ~46.9 us pure-DMA floor
(358 GB/s HBM-per-core on 16 MiB of traffic). Engine budgets under the floor:
PE ~25 us, DVE ~25 us, ACT ~23 us. rel_err vs f32 reference ~4.3e-3 (gate
2e-2; bf16 input quantization + bf16 carry-chain rounding).
"""

import numpy as np

B, L, D, N = 8, 4096, 64, 16
F = D * N            # 1024 features per batch
NCORES = 8
LBLK = 128           # L positions per matmul block
M = 2                # L-blocks per PSUM window (vector-op granularity)
NGROUP = F // 128    # 8 feature groups
NBLK = L // LBLK     # 32 L-blocks
NWIN = NBLK // M     # 16 windows
CBLK = 2             # L-blocks per input DMA chunk (512 KiB)
SPAN = 2048          # L columns per staged output tile (512 KiB out-DMAs)
WIN_PER_SPAN = SPAN // (M * LBLK)
XIN_BUFS = 6

_CACHE = {}


def _build_nc(loop_nrep=None):
    """Build the Bass program. loop_nrep wraps the body in a device-side For_i -
    used only by test.py for timing (the graded path uses loop_nrep=None)."""
    from contextlib import nullcontext

    import concourse.bacc as bacc
    import concourse.mybir as mybir
    from concourse.tile import TileContext

    f32 = mybir.dt.float32
    bf16 = mybir.dt.bfloat16
    nc = bacc.Bacc(
        "TRN2", target_bir_lowering=False, debug=False, num_devices=NCORES
    )
    x = nc.dram_tensor("x", (L, F), bf16, kind="ExternalInput")
    u = nc.dram_tensor("u", (LBLK, M * LBLK), bf16, kind="ExternalInput")
    y = nc.dram_tensor("y", (F, L), bf16, kind="ExternalOutput")

    with TileContext(nc) as tc:
        with (
            tc.tile_pool(name="const", bufs=1) as cpool,
            tc.tile_pool(name="xin", bufs=XIN_BUFS) as xpool,
            tc.tile_pool(name="stage", bufs=2) as spool,
            tc.tile_pool(name="psum", bufs=8, space="PSUM") as ppool,
        ):
            ut = cpool.tile([LBLK, M * LBLK], bf16)
            nc.sync.dma_start(out=ut[:], in_=u[:, :])
            zt = cpool.tile([128, 1], bf16)
            nc.vector.memset(zt[:], 0.0)

            loop_cm = tc.For_i(0, loop_nrep, 1) if loop_nrep else nullcontext()
            loop_cm.__enter__()
            staged = [None] * NGROUP
            prev_staged = [None] * NGROUP
            for ci in range(NBLK // CBLK):  # 512 KiB input chunks: 2 L-blocks
                xt = xpool.tile([128, CBLK * F], bf16, tag="xt", name=f"xt_{ci}")
                nc.sync.dma_start(
                    out=xt[:],
                    in_=x[ci * (CBLK * 128) : (ci + 1) * (CBLK * 128), :].rearrange(
                        "(t p) f -> p t f", p=128
                    ),
                )
                for wi in range(CBLK // M):
                    w = ci * (CBLK // M) + wi
                    s, wb = divmod(w, WIN_PER_SPAN)
                    for g in range(NGROUP):
                        if wb == 0:
                            prev_staged[g] = staged[g]
                            staged[g] = spool.tile(
                                [128, SPAN], bf16, tag=f"st{g}", name=f"st{g}_{s}"
                            )
                        ps = ppool.tile(
                            [128, M * LBLK], f32, tag="ps", name=f"ps_{w}_{g}"
                        )
                        for j in range(M):
                            nc.tensor.matmul(
                                ps[:, j * LBLK :],
                                xt[
                                    :,
                                    (wi * M + j) * F
                                    + g * 128 : (wi * M + j) * F
                                    + (g + 1) * 128,
                                ],
                                ut[:, : (M - j) * LBLK],
                                start=(j == 0),
                                stop=(j == M - 1),
                            )
                        dst = staged[g][:, wb * M * LBLK : (wb + 1) * M * LBLK]
                        if w == 0:
                            carry = zt[:]
                        elif wb > 0:
                            carry = staged[g][:, wb * M * LBLK - 1 : wb * M * LBLK]
                        else:
                            carry = prev_staged[g][:, SPAN - 1 : SPAN]
                        if g < NGROUP // 2:
                            nc.vector.tensor_tensor(
                                out=dst,
                                in0=ps[:],
                                in1=carry.broadcast_to([128, M * LBLK]),
                                op=mybir.AluOpType.add,
                            )
                        else:
                            nc.scalar.add(out=dst, in_=ps[:], add=carry)
                        if wb == WIN_PER_SPAN - 1:
                            nc.sync.dma_start(
                                out=y[
                                    g * 128 : (g + 1) * 128, s * SPAN : (s + 1) * SPAN
                                ],
                                in_=staged[g][:],
                            )
            loop_cm.__exit__(None, None, None)
    nc.compile()
    return nc


def _get_nc():
    if "nc" not in _CACHE:
        _CACHE["nc"] = _build_nc()
    return _CACHE["nc"]


def _make_in_maps(X_in):
    import ml_dtypes

    bf16 = ml_dtypes.bfloat16
    xs = np.ascontiguousarray(np.asarray(X_in, dtype=np.float32)).reshape(B, L, F)
    xs = xs.astype(bf16)
    umat = np.concatenate(
        [np.triu(np.ones((LBLK, LBLK), dtype=np.float32))]
        + [np.ones((LBLK, LBLK), dtype=np.float32)] * (M - 1),
        axis=1,
    ).astype(bf16)
    return [{"x": xs[b], "u": umat} for b in range(B)]


def _unshard(per_core_outs):
    out = np.empty((B, L, D, N), dtype=np.float32)
    for b in range(B):
        out[b] = np.asarray(per_core_outs[b]["y"], dtype=np.float32).T.reshape(L, D, N)
    return out


def kernel(X_in):
    from concourse.bass_utils import run_bass_kernel_spmd

    nc = _get_nc()
    in_maps = _make_in_maps(X_in)
    try:
        res = run_bass_kernel_spmd(nc, in_maps, core_ids=list(range(NCORES)))
        out = _unshard(res.results)
    except Exception:
        # Transient device wedges (NRT_EXEC_UNIT_UNRECOVERABLE) have been
        # observed on this axon path; one retry recovers them.
        res = run_bass_kernel_spmd(nc, in_maps, core_ids=list(range(NCORES)))
        out = _unshard(res.results)
    return out


# revision 12
# speedup vs baseline: 1.0516x; 1.0295x over previous
"""Inclusive prefix-sum (Blelloch scan, additive) along L for X_in (8, 4096, 64, 16) f32.

Sharding: batch B=8 across the 8 NeuronCores (one batch per core; no communication).
Per core the problem is a cumsum along L=4096 of a (L, F=1024) matrix.

Per-core kernel ("transposed-output matmul scan", bf16 I/O):
  - The rel-err gate is 2e-2, so device I/O is bf16 (host converts f32<->bf16):
    16 MiB of HBM traffic per core instead of 32 MiB -> ~47 us DMA floor.
  - L is processed in M=2-block (256-row) windows. Per window and 128-wide
    feature group, 3 bf16 matmuls accumulate into one PSUM tile [128, 256]:
        mm0: lhsT=x_b0, rhs=[triu|ones]  -> cols 0..255   (start)
        mm1: lhsT=x_b1, rhs= triu        -> cols 128..255 (accumulate, stop)
    giving the within-window inclusive scan transposed (features on partitions),
    accumulated in f32 PSUM.
  - The inter-window carry is a per-partition scalar fused into the PSUM->SBUF
    copy (DVE tensor_tensor with a stride-0 broadcast carry for groups 0-3 —
    DVE tensor_scalar requires f32 scalars — ACT activation bias for groups
    4-7), which also converts f32->bf16. Carry for window w is the last
    already-written column of the staged bf16 output; window 0 uses a zeros
    column so every window runs the identical op.
  - Input DMA: 512 KiB chunks (2 L-blocks) on the sync HWDGE ring (smaller
    first-chunk ramp measured ~0.8 us faster than 1 MiB chunks). Output staged
    in (128, 1024) bf16 tiles -> 256 KiB out-DMAs, also on sync (the shorter
    final-DMA tail measured ~0.8 us faster than 2048-col spans).
  - Output is written transposed, y (F, L) bf16; numpy un-transposes and
    upcasts when unsharding.

Measured (For_i loop-diff on HW, 8 cores concurrent): ~61 us/iteration incl.
~13.5 us For_i back-edge; ~47.4 us single-shot, vs ~46.9 us pure-DMA floor
(358 GB/s HBM-per-core on 16 MiB of traffic). Engine budgets under the floor:
PE ~25 us, DVE ~25 us, ACT ~23 us. rel_err vs f32 reference ~4.3e-3 (gate
2e-2; bf16 input quantization + bf16 carry-chain rounding).
"""

import numpy as np

B, L, D, N = 8, 4096, 64, 16
F = D * N            # 1024 features per batch
NCORES = 8
LBLK = 128           # L positions per matmul block
M = 2                # L-blocks per PSUM window (vector-op granularity)
NGROUP = F // 128    # 8 feature groups
NBLK = L // LBLK     # 32 L-blocks
NWIN = NBLK // M     # 16 windows
CBLK = 2             # L-blocks per input DMA chunk (512 KiB)
SPAN = 1024          # L columns per staged output tile (256 KiB out-DMAs)
WIN_PER_SPAN = SPAN // (M * LBLK)
XIN_BUFS = 6

_CACHE = {}


def _build_nc(loop_nrep=None):
    """Build the Bass program. loop_nrep wraps the body in a device-side For_i -
    used only by test.py for timing (the graded path uses loop_nrep=None)."""
    from contextlib import nullcontext

    import concourse.bacc as bacc
    import concourse.mybir as mybir
    from concourse.tile import TileContext

    f32 = mybir.dt.float32
    bf16 = mybir.dt.bfloat16
    nc = bacc.Bacc(
        "TRN2", target_bir_lowering=False, debug=False, num_devices=NCORES
    )
    x = nc.dram_tensor("x", (L, F), bf16, kind="ExternalInput")
    u = nc.dram_tensor("u", (LBLK, M * LBLK), bf16, kind="ExternalInput")
    y = nc.dram_tensor("y", (F, L), bf16, kind="ExternalOutput")

    with TileContext(nc) as tc:
        with (
            tc.tile_pool(name="const", bufs=1) as cpool,
            tc.tile_pool(name="xin", bufs=XIN_BUFS) as xpool,
            tc.tile_pool(name="stage", bufs=2) as spool,
            tc.tile_pool(name="psum", bufs=8, space="PSUM") as ppool,
        ):
            ut = cpool.tile([LBLK, M * LBLK], bf16)
            nc.sync.dma_start(out=ut[:], in_=u[:, :])
            zt = cpool.tile([128, 1], bf16)
            nc.vector.memset(zt[:], 0.0)

            loop_cm = tc.For_i(0, loop_nrep, 1) if loop_nrep else nullcontext()
            loop_cm.__enter__()
            staged = [None] * NGROUP
            prev_staged = [None] * NGROUP
            for ci in range(NBLK // CBLK):  # 512 KiB input chunks: 2 L-blocks
                xt = xpool.tile([128, CBLK * F], bf16, tag="xt", name=f"xt_{ci}")
                nc.sync.dma_start(
                    out=xt[:],
                    in_=x[ci * (CBLK * 128) : (ci + 1) * (CBLK * 128), :].rearrange(
                        "(t p) f -> p t f", p=128
                    ),
                )
                for wi in range(CBLK // M):
                    w = ci * (CBLK // M) + wi
                    s, wb = divmod(w, WIN_PER_SPAN)
                    for g in range(NGROUP):
                        if wb == 0:
                            prev_staged[g] = staged[g]
                            staged[g] = spool.tile(
                                [128, SPAN], bf16, tag=f"st{g}", name=f"st{g}_{s}"
                            )
                        ps = ppool.tile(
                            [128, M * LBLK], f32, tag="ps", name=f"ps_{w}_{g}"
                        )
                        for j in range(M):
                            nc.tensor.matmul(
                                ps[:, j * LBLK :],
                                xt[
                                    :,
                                    (wi * M + j) * F
                                    + g * 128 : (wi * M + j) * F
                                    + (g + 1) * 128,
                                ],
                                ut[:, : (M - j) * LBLK],
                                start=(j == 0),
                                stop=(j == M - 1),
                            )
                        dst = staged[g][:, wb * M * LBLK : (wb + 1) * M * LBLK]
                        if w == 0:
                            carry = zt[:]
                        elif wb > 0:
                            carry = staged[g][:, wb * M * LBLK - 1 : wb * M * LBLK]
                        else:
                            carry = prev_staged[g][:, SPAN - 1 : SPAN]
                        if g < NGROUP // 2:
                            nc.vector.tensor_tensor(
                                out=dst,
                                in0=ps[:],
                                in1=carry.broadcast_to([128, M * LBLK]),
                                op=mybir.AluOpType.add,
                            )
                        else:
                            nc.scalar.add(out=dst, in_=ps[:], add=carry)
                        if wb == WIN_PER_SPAN - 1:
                            nc.sync.dma_start(
                                out=y[
                                    g * 128 : (g + 1) * 128, s * SPAN : (s + 1) * SPAN
                                ],
                                in_=staged[g][:],
                            )
            loop_cm.__exit__(None, None, None)
    nc.compile()
    return nc


def _get_nc():
    if "nc" not in _CACHE:
        _CACHE["nc"] = _build_nc()
    return _CACHE["nc"]


def _make_in_maps(X_in):
    import ml_dtypes

    bf16 = ml_dtypes.bfloat16
    xs = np.ascontiguousarray(np.asarray(X_in, dtype=np.float32)).reshape(B, L, F)
    xs = xs.astype(bf16)
    umat = np.concatenate(
        [np.triu(np.ones((LBLK, LBLK), dtype=np.float32))]
        + [np.ones((LBLK, LBLK), dtype=np.float32)] * (M - 1),
        axis=1,
    ).astype(bf16)
    return [{"x": xs[b], "u": umat} for b in range(B)]


def _unshard(per_core_outs):
    out = np.empty((B, L, D, N), dtype=np.float32)
    for b in range(B):
        out[b] = np.asarray(per_core_outs[b]["y"], dtype=np.float32).T.reshape(L, D, N)
    return out


def kernel(X_in):
    from concourse.bass_utils import run_bass_kernel_spmd

    nc = _get_nc()
    in_maps = _make_in_maps(X_in)
    try:
        res = run_bass_kernel_spmd(nc, in_maps, core_ids=list(range(NCORES)))
        out = _unshard(res.results)
    except Exception:
        # Transient device wedges (NRT_EXEC_UNIT_UNRECOVERABLE) have been
        # observed on this axon path; one retry recovers them.
        res = run_bass_kernel_spmd(nc, in_maps, core_ids=list(range(NCORES)))
        out = _unshard(res.results)
    return out
